# revision 9
# baseline (speedup 1.0000x reference)
"""Trainium2 Bass kernel for NeuromorphicLM (8 NeuronCores, token-sharded SPMD).

Sharding: the 4096 tokens (BS*N flattened) are split into 8 contiguous shards of
512 tokens; every core runs the full pipeline (embed gather -> fan_out -> 3
refine passes -> fan_in -> LayerNorm -> tied lm_head over the whole 32k vocab)
on its shard. No collectives. Host gathers logits shards and sums pcm partials.
"""
import sys

sys.path.insert(0, "/opt/trn_rl_repo")

from contextlib import ExitStack

import numpy as np
import ml_dtypes

import concourse.bass as bass
import concourse.bacc as bacc
import concourse.tile as tile
from concourse import mybir
from concourse import bass_utils
from concourse.masks import make_identity

BF = ml_dtypes.bfloat16
F32 = mybir.dt.float32
BF16 = mybir.dt.bfloat16

BS, N, V, D = 4, 1024, 32000, 1024
B, C, Dc, H, R = 8, 4, 64, 256, 3
T = B * C * Dc            # 2048
NG = B * C                # 32 (block, channel) groups of 64 features
NCORES = 8
NTOK = (BS * N) // NCORES  # 512 tokens per core
NTT = NTOK // 128          # 4 token tiles of 128
VCH = 500                  # vocab chunk (lm_head matmul N)
NVC = V // VCH             # 64 chunks
PCM_W = 0.1
LN_EPS = 1e-5

AF = mybir.ActivationFunctionType
ALU = mybir.AluOpType

_PROGRAM = None
LAST_RESULT = None


def _build():
    # Bacc (not plain Bass): its finalize() runs the backend passes, notably
    # generate_event_semaphores which splits multi-sem waits (TRN2 allows one
    # sync wait per instruction).
    nc = bacc.Bacc("TRN2")

    ids_h = nc.declare_dram_parameter("ids", [128, NTT], mybir.dt.int32, isOutput=False)
    emb_h = nc.declare_dram_parameter("emb", [V, D], BF16, isOutput=False)
    et_h = nc.declare_dram_parameter("et", [D, V], BF16, isOutput=False)
    pos_h = nc.declare_dram_parameter("pos", [NTOK, D], F32, isOutput=False)
    fow_h = nc.declare_dram_parameter("fo_w", [128, 8, T], BF16, isOutput=False)
    fob_h = nc.declare_dram_parameter("fo_b", [64, NG], F32, isOutput=False)
    w1_h = nc.declare_dram_parameter("w1", [64, B, H], BF16, isOutput=False)
    w2a_h = nc.declare_dram_parameter("w2a", [128, B, 2, 64], BF16, isOutput=False)
    w2b_h = nc.declare_dram_parameter("w2b", [128, B, 2, 64], BF16, isOutput=False)
    wz_h = nc.declare_dram_parameter("wz", [64, B, 64], BF16, isOutput=False)
    fiw_h = nc.declare_dram_parameter("fi_w", [64, NG, D], BF16, isOutput=False)
    fib_h = nc.declare_dram_parameter("fi_b", [1, D], F32, isOutput=False)
    lng_h = nc.declare_dram_parameter("ln_g", [1, D], F32, isOutput=False)
    lnb_h = nc.declare_dram_parameter("ln_b", [1, D], F32, isOutput=False)
    logits_h = nc.declare_dram_parameter("logits", [NTOK, V], F32, isOutput=True)
    aux_h = nc.declare_dram_parameter("aux", [2, 64, B], F32, isOutput=True)

    with tile.TileContext(nc) as tc, ExitStack() as ctx:
        const = ctx.enter_context(tc.tile_pool(name="const", bufs=1))
        ident = const.tile([128, 128], BF16)
        make_identity(nc, ident[:])

        ids_sb = const.tile([128, NTT], mybir.dt.int32)
        nc.sync.dma_start(out=ids_sb[:], in_=ids_h[:])
        fob_sb = const.tile([64, NG], F32)
        nc.sync.dma_start(out=fob_sb[:], in_=fob_h[:])
        w1_sb = const.tile([64, B, H], BF16)
        nc.sync.dma_start(out=w1_sb[:], in_=w1_h[:])
        w2a_sb = const.tile([128, B, 2, 64], BF16)
        nc.sync.dma_start(out=w2a_sb[:], in_=w2a_h[:])
        w2b_sb = const.tile([128, B, 2, 64], BF16)
        nc.sync.dma_start(out=w2b_sb[:], in_=w2b_h[:])
        wz_sb = const.tile([64, B, 64], BF16)
        nc.sync.dma_start(out=wz_sb[:], in_=wz_h[:])
        fib_bc = const.tile([128, D], F32)
        nc.sync.dma_start(out=fib_bc[:], in_=fib_h[:].to_broadcast([128, D]))
        lng_bc = const.tile([128, D], F32)
        nc.sync.dma_start(out=lng_bc[:], in_=lng_h[:].to_broadcast([128, D]))
        lnb_bc = const.tile([128, D], F32)
        nc.sync.dma_start(out=lnb_bc[:], in_=lnb_h[:].to_broadcast([128, D]))
        eps_sb = const.tile([128, 1], F32)
        nc.vector.memset(eps_sb[:], LN_EPS)

        # long-lived activations (reserved early; lifetimes span several phases)
        late = ctx.enter_context(tc.tile_pool(name="late", bufs=1))
        xbf_sb = late.tile([64, NG, NTOK], BF16)     # final-x bf16 (written in pass r=2)
        yn_all = late.tile([128, NTT, D], BF16)      # post-LN, token-major
        ynT = late.tile([128, 8, NTOK], BF16)        # post-LN, feature-major

        with tc.tile_pool(name="xmaster", bufs=1) as xpool:
            x_sb = xpool.tile([64, NG, NTOK], F32)   # f32 master, per-group [64, 512]

            # ---- Phase 1: embedding gather + pos add + transpose to feature-major
            with tc.tile_pool(name="ph1", bufs=2) as ph1, \
                 tc.tile_pool(name="ph1xt", bufs=1) as ph1xt, \
                 tc.tile_pool(name="ph1w", bufs=1) as ph1w, \
                 tc.tile_pool(name="ph1ps", bufs=2, space="PSUM") as ph1ps, \
                 tc.tile_pool(name="fops", bufs=4, space="PSUM") as fops:
                xT = ph1xt.tile([128, 8, NTOK], BF16)    # x^T: [d-part, d-tile, tok]
                # single gather target: SWDGE pseudo-DMAs only support one sync
                # wait, so avoid slot-reuse WAR deps on the gather output
                g_all = ph1xt.tile([128, NTT, D], BF16)
                for tb in range(NTT):
                    nc.gpsimd.indirect_dma_start(
                        out=g_all[:, tb, :],
                        out_offset=None,
                        in_=emb_h[:],
                        in_offset=bass.IndirectOffsetOnAxis(ap=ids_sb[:, tb:tb + 1], axis=0),
                    )
                for tb in range(NTT):
                    p_tile = ph1.tile([128, D], F32)
                    nc.sync.dma_start(out=p_tile[:], in_=pos_h[tb * 128:(tb + 1) * 128, :])
                    xe = ph1.tile([128, D], F32)
                    nc.scalar.copy(out=xe[:], in_=g_all[:, tb, :])
                    xtok = ph1.tile([128, D], BF16)
                    nc.vector.tensor_add(out=xtok[:], in0=xe[:], in1=p_tile[:])
                    for dt in range(8):
                        tp = ph1ps.tile([128, 128], BF16)
                        nc.tensor.transpose(out=tp[:], in_=xtok[:, dt * 128:(dt + 1) * 128],
                                            identity=ident[:])
                        nc.scalar.copy(out=xT[:, dt, tb * 128:(tb + 1) * 128], in_=tp[:])

                # ---- Phase 2: fan_out  x_g = x @ fo_w[:, g] + fo_b[g]
                for half in range(2):
                    fow_sb = ph1w.tile([128, 8, T // 2], BF16)
                    nc.sync.dma_start(out=fow_sb[:],
                                      in_=fow_h[:, :, half * (T // 2):(half + 1) * (T // 2)])
                    for lg in range(NG // 2):
                        g = half * (NG // 2) + lg
                        ps = fops.tile([64, NTOK], F32)
                        for kt in range(8):
                            nc.tensor.matmul(
                                out=ps[:],
                                lhsT=fow_sb[:, kt, lg * 64:(lg + 1) * 64],
                                rhs=xT[:, kt, :],
                                start=(kt == 0), stop=(kt == 7),
                            )
                        nc.vector.tensor_scalar(
                            out=x_sb[:, g, :], in0=ps[:],
                            scalar1=fob_sb[:, g:g + 1], scalar2=None, op0=ALU.add,
                        )

            # ---- Phase 3: refine passes
            with tc.tile_pool(name="zstate", bufs=1) as zstate, \
                 tc.tile_pool(name="zp", bufs=2) as zp, \
                 tc.tile_pool(name="zbfp", bufs=2) as zbfp, \
                 tc.tile_pool(name="xbfp", bufs=4) as xbfp, \
                 tc.tile_pool(name="hbp", bufs=4) as hbp, \
                 tc.tile_pool(name="hps", bufs=3, space="PSUM") as hps, \
                 tc.tile_pool(name="ups", bufs=2, space="PSUM") as ups, \
                 tc.tile_pool(name="zps", bufs=2, space="PSUM") as zps:
                # zhat4 holds 4*z_hat (Wz is pre-scaled by 4 host-side) so the
                # pcm diff can use the un-normalized group sum directly.
                zhat4 = zstate.tile([64, B, NTOK], F32)
                acc1 = zstate.tile([64, B], F32)
                acc2 = zstate.tile([64, B], F32)
                nc.vector.memset(acc1[:], 0.0)
                nc.vector.memset(acc2[:], 0.0)
                for r in range(R):
                    # z-path (reads pre-update x)
                    for b in range(B):
                        zsum = zp.tile([64, NTOK], F32)
                        nc.vector.tensor_add(out=zsum[:], in0=x_sb[:, 4 * b, :],
                                             in1=x_sb[:, 4 * b + 1, :])
                        nc.vector.tensor_add(out=zsum[:], in0=zsum[:], in1=x_sb[:, 4 * b + 2, :])
                        nc.vector.tensor_add(out=zsum[:], in0=zsum[:], in1=x_sb[:, 4 * b + 3, :])
                        if r >= 1:
                            accr = acc1 if r == 1 else acc2
                            dd = zp.tile([64, NTOK], F32)
                            nc.vector.tensor_tensor(out=dd[:], in0=zsum[:], in1=zhat4[:, b, :],
                                                    op=ALU.subtract)
                            dd2 = zp.tile([64, NTOK], F32)
                            nc.scalar.activation(out=dd2[:], in_=dd[:], func=AF.Square,
                                                 accum_out=accr[:, b:b + 1])
                        if r <= 1:
                            zb = zbfp.tile([64, NTOK], BF16)
                            nc.scalar.mul(out=zb[:], in_=zsum[:], mul=0.25)
                            zph = zps.tile([64, NTOK], F32)
                            nc.tensor.matmul(out=zph[:], lhsT=wz_sb[:, b, :], rhs=zb[:],
                                             start=True, stop=True)
                            nc.scalar.copy(out=zhat4[:, b, :], in_=zph[:])
                    # MLP path
                    w2_sb = w2a_sb if r == 0 else w2b_sb
                    for g in range(NG):
                        b = g // 4
                        xb = xbfp.tile([64, NTOK], BF16)
                        nc.gpsimd.tensor_copy(out=xb[:], in_=x_sb[:, g, :])
                        hts = []
                        for hh in range(2):
                            hp = hps.tile([128, NTOK], F32)
                            nc.tensor.matmul(out=hp[:], lhsT=w1_sb[:, b, hh * 128:(hh + 1) * 128],
                                             rhs=xb[:], start=True, stop=True)
                            ht = hbp.tile([128, NTOK], BF16)
                            nc.scalar.activation(out=ht[:], in_=hp[:], func=AF.Gelu_apprx_tanh)
                            hts.append(ht)
                        up = ups.tile([64, NTOK], F32)
                        for kk in range(2):
                            nc.tensor.matmul(out=up[:], lhsT=w2_sb[:, b, kk, :], rhs=hts[kk][:],
                                             start=(kk == 0), stop=(kk == 1))
                        if r < R - 1:
                            nc.vector.tensor_add(out=x_sb[:, g, :], in0=x_sb[:, g, :], in1=up[:])
                        else:
                            nc.vector.tensor_add(out=xbf_sb[:, g, :], in0=x_sb[:, g, :], in1=up[:])
                nc.sync.dma_start(out=aux_h[0], in_=acc1[:])
                nc.sync.dma_start(out=aux_h[1], in_=acc2[:])

        # x_sb freed here (xpool closed)

        # ---- Phase 4: fan_in (token-major out) + LayerNorm
        with tc.tile_pool(name="fiw", bufs=1) as fiwp, \
             tc.tile_pool(name="yp", bufs=1) as yp, \
             tc.tile_pool(name="lnp", bufs=4) as lnp, \
             tc.tile_pool(name="fips", bufs=4, space="PSUM") as fips:
            fiw_sb = fiwp.tile([64, NG, D], BF16)
            nc.sync.dma_start(out=fiw_sb[:], in_=fiw_h[:])
            y_all = yp.tile([128, NTT, D], F32)
            for tt in range(NTT):
                for half in range(2):
                    ps = fips.tile([128, 512], F32)
                    for g in range(NG):
                        nc.tensor.matmul(
                            out=ps[:],
                            lhsT=xbf_sb[:, g, tt * 128:(tt + 1) * 128],
                            rhs=fiw_sb[:, g, half * 512:(half + 1) * 512],
                            start=(g == 0), stop=(g == NG - 1),
                        )
                    nc.vector.tensor_add(
                        out=y_all[:, tt, half * 512:(half + 1) * 512],
                        in0=ps[:], in1=fib_bc[:, half * 512:(half + 1) * 512],
                    )
            nbs = nc.vector.BN_STATS_DIM
            for tt in range(NTT):
                st = lnp.tile([128, 2, nbs], F32)
                nc.vector.bn_stats(out=st[:, 0, :], in_=y_all[:, tt, 0:512])
                nc.vector.bn_stats(out=st[:, 1, :], in_=y_all[:, tt, 512:1024])
                mv = lnp.tile([128, nc.vector.BN_AGGR_DIM], F32)
                nc.vector.bn_aggr(out=mv[:], in_=st[:])
                sd = lnp.tile([128, 1], F32)
                nc.scalar.activation(out=sd[:], in_=mv[:, 1:2], func=AF.Sqrt, bias=eps_sb[:])
                rstd = lnp.tile([128, 1], F32)
                nc.vector.reciprocal(out=rstd[:], in_=sd[:])
                tmp = lnp.tile([128, D], F32)
                nc.vector.tensor_scalar(out=tmp[:], in0=y_all[:, tt, :],
                                        scalar1=mv[:, 0:1], scalar2=rstd[:],
                                        op0=ALU.subtract, op1=ALU.mult)
                nc.vector.tensor_mul(out=tmp[:], in0=tmp[:], in1=lng_bc[:])
                nc.vector.tensor_add(out=yn_all[:, tt, :], in0=tmp[:], in1=lnb_bc[:])

        # ---- Phase 5: transpose yn to feature-major
        with tc.tile_pool(name="tp2", bufs=2, space="PSUM") as tp2:
            for tt in range(NTT):
                for dt in range(8):
                    tp = tp2.tile([128, 128], BF16)
                    nc.tensor.transpose(out=tp[:], in_=yn_all[:, tt, dt * 128:(dt + 1) * 128],
                                        identity=ident[:])
                    nc.scalar.copy(out=ynT[:, dt, tt * 128:(tt + 1) * 128], in_=tp[:])

        # ---- Phase 6: lm_head  logits = yn @ E^T
        et_ap = et_h[:].rearrange("(dt p) v -> p dt v", p=128)
        with tc.tile_pool(name="slab", bufs=3) as slabp, \
             tc.tile_pool(name="lst", bufs=4) as lstp, \
             tc.tile_pool(name="lmps", bufs=4, space="PSUM") as lmps:
            for vc in range(NVC):
                slab = slabp.tile([128, 8, VCH], BF16)
                nc.sync.dma_start(out=slab[:], in_=et_ap[:, :, vc * VCH:(vc + 1) * VCH])
                for tt in range(NTT):
                    ps = lmps.tile([128, VCH], F32)
                    for dt in range(8):
                        nc.tensor.matmul(
                            out=ps[:],
                            lhsT=ynT[:, dt, tt * 128:(tt + 1) * 128],
                            rhs=slab[:, dt, :],
                            start=(dt == 0), stop=(dt == 7),
                        )
                    ls = lstp.tile([128, VCH], F32)
                    nc.scalar.copy(out=ls[:], in_=ps[:])
                    nc.scalar.dma_start(
                        out=logits_h[tt * 128:(tt + 1) * 128, vc * VCH:(vc + 1) * VCH],
                        in_=ls[:],
                    )
    nc.finalize()
    return nc


def _get_program():
    global _PROGRAM
    if _PROGRAM is None:
        _PROGRAM = _build()
    return _PROGRAM


def kernel(**inputs):
    global LAST_RESULT
    nc = _get_program()

    ids = np.ascontiguousarray(np.asarray(inputs["input_ids"], np.int32).reshape(-1))
    emb = np.asarray(inputs["embedding"], np.float32)
    pos = np.asarray(inputs["pos_emb"], np.float32)
    lam = float(1.0 / (1.0 + np.exp(-np.float64(inputs["lambda_logit"]))))

    emb_bf = np.ascontiguousarray(emb.astype(BF))
    et_bf = np.ascontiguousarray(emb.T).astype(BF)
    fo_w = np.ascontiguousarray(
        np.asarray(inputs["fo_w"], np.float32).reshape(8, 128, T).transpose(1, 0, 2)
    ).astype(BF)
    fo_b = np.ascontiguousarray(
        np.asarray(inputs["fo_b"], np.float32).reshape(NG, 64).T
    )
    w1 = np.ascontiguousarray(
        np.asarray(inputs["W1"], np.float32).transpose(1, 0, 2)
    ).astype(BF)
    w2f = np.asarray(inputs["W2"], np.float32)
    w2a = np.ascontiguousarray(
        w2f.reshape(B, 2, 128, 64).transpose(2, 0, 1, 3)
    ).astype(BF)
    w2b = np.ascontiguousarray(
        (lam * w2f).reshape(B, 2, 128, 64).transpose(2, 0, 1, 3)
    ).astype(BF)
    wz = np.ascontiguousarray(
        (4.0 * np.asarray(inputs["Wz"], np.float32)).transpose(1, 0, 2)
    ).astype(BF)
    fi_w = np.ascontiguousarray(
        np.asarray(inputs["fi_w"], np.float32).reshape(NG, 64, D).transpose(1, 0, 2)
    ).astype(BF)
    fi_b = np.ascontiguousarray(np.asarray(inputs["fi_b"], np.float32).reshape(1, D))
    ln_g = np.ascontiguousarray(np.asarray(inputs["ln_g"], np.float32).reshape(1, D))
    ln_b = np.ascontiguousarray(np.asarray(inputs["ln_b"], np.float32).reshape(1, D))

    in_maps = []
    for i in range(NCORES):
        ids_i = np.ascontiguousarray(
            ids[i * NTOK:(i + 1) * NTOK].reshape(NTT, 128).T
        )
        p0 = (i * NTOK) % N
        pos_i = np.ascontiguousarray(pos[p0:p0 + NTOK])
        in_maps.append({
            "ids": ids_i, "emb": emb_bf, "et": et_bf, "pos": pos_i,
            "fo_w": fo_w, "fo_b": fo_b, "w1": w1, "w2a": w2a, "w2b": w2b,
            "wz": wz, "fi_w": fi_w, "fi_b": fi_b, "ln_g": ln_g, "ln_b": ln_b,
        })

    br = bass_utils.run_bass_kernel_spmd(nc, in_maps, core_ids=list(range(NCORES)))
    LAST_RESULT = br

    logits = np.concatenate(
        [np.asarray(r["logits"], np.float32) for r in br.results], axis=0
    ).reshape(BS, N, V)
    s = np.float64(0.0)
    for r in br.results:
        s += np.asarray(r["aux"], np.float64).sum()
    # acc holds sum((4z - 4zhat)^2) = 16 * sum((z - zhat)^2)
    aux = np.float32(PCM_W * (s / 16.0) / (BS * N * Dc))
    return logits, aux


# revision 16
# speedup vs baseline: 1.0067x; 1.0067x over previous
"""Trainium2 Bass kernel for NeuromorphicLM (8 NeuronCores, token-sharded SPMD).

Sharding: the 4096 tokens (BS*N flattened) are split into 8 contiguous shards of
512 tokens; every core runs the full pipeline (embed gather -> fan_out -> 3
refine passes -> fan_in -> LayerNorm -> tied lm_head over the whole 32k vocab)
on its shard. No collectives. Host gathers logits shards and sums pcm partials.
"""
import sys

sys.path.insert(0, "/opt/trn_rl_repo")

from contextlib import ExitStack

import numpy as np
import ml_dtypes

import concourse.bass as bass
import concourse.bacc as bacc
import concourse.tile as tile
from concourse import mybir
from concourse import bass_utils
from concourse.masks import make_identity

BF = ml_dtypes.bfloat16
F32 = mybir.dt.float32
BF16 = mybir.dt.bfloat16

BS, N, V, D = 4, 1024, 32000, 1024
B, C, Dc, H, R = 8, 4, 64, 256, 3
T = B * C * Dc            # 2048
NG = B * C                # 32 (block, channel) groups of 64 features
NCORES = 8
NTOK = (BS * N) // NCORES  # 512 tokens per core
NTT = NTOK // 128          # 4 token tiles of 128
VCH = 500                  # vocab chunk (lm_head matmul N)
NVC = V // VCH             # 64 chunks
PCM_W = 0.1
LN_EPS = 1e-5

AF = mybir.ActivationFunctionType
ALU = mybir.AluOpType

_PROGRAM = None
LAST_RESULT = None


def _build():
    # Bacc (not plain Bass): its finalize() runs the backend passes, notably
    # generate_event_semaphores which splits multi-sem waits (TRN2 allows one
    # sync wait per instruction).
    nc = bacc.Bacc("TRN2")

    ids_h = nc.declare_dram_parameter("ids", [128, NTT], mybir.dt.int32, isOutput=False)
    emb_h = nc.declare_dram_parameter("emb", [V, D], BF16, isOutput=False)
    et_h = nc.declare_dram_parameter("et", [D, V], BF16, isOutput=False)
    pos_h = nc.declare_dram_parameter("pos", [NTOK, D], F32, isOutput=False)
    fow_h = nc.declare_dram_parameter("fo_w", [128, 8, T], BF16, isOutput=False)
    fob_h = nc.declare_dram_parameter("fo_b", [64, NG], F32, isOutput=False)
    w1_h = nc.declare_dram_parameter("w1", [64, B, H], BF16, isOutput=False)
    w2a_h = nc.declare_dram_parameter("w2a", [128, B, 2, 64], BF16, isOutput=False)
    w2b_h = nc.declare_dram_parameter("w2b", [128, B, 2, 64], BF16, isOutput=False)
    wz_h = nc.declare_dram_parameter("wz", [64, B, 64], BF16, isOutput=False)
    fiw_h = nc.declare_dram_parameter("fi_w", [64, NG, D], BF16, isOutput=False)
    fib_h = nc.declare_dram_parameter("fi_b", [1, D], F32, isOutput=False)
    lng_h = nc.declare_dram_parameter("ln_g", [1, D], F32, isOutput=False)
    lnb_h = nc.declare_dram_parameter("ln_b", [1, D], F32, isOutput=False)
    logits_h = nc.declare_dram_parameter("logits", [NTOK, V], F32, isOutput=True)
    aux_h = nc.declare_dram_parameter("aux", [2, 64, B], F32, isOutput=True)

    with tile.TileContext(nc) as tc, ExitStack() as ctx:
        const = ctx.enter_context(tc.tile_pool(name="const", bufs=1))
        ident = const.tile([128, 128], BF16)
        make_identity(nc, ident[:])

        ids_sb = const.tile([128, NTT], mybir.dt.int32)
        nc.sync.dma_start(out=ids_sb[:], in_=ids_h[:])
        fob_sb = const.tile([64, NG], F32)
        nc.sync.dma_start(out=fob_sb[:], in_=fob_h[:])
        w1_sb = const.tile([64, B, H], BF16)
        nc.sync.dma_start(out=w1_sb[:], in_=w1_h[:])
        w2a_sb = const.tile([128, B, 2, 64], BF16)
        nc.sync.dma_start(out=w2a_sb[:], in_=w2a_h[:])
        w2b_sb = const.tile([128, B, 2, 64], BF16)
        nc.sync.dma_start(out=w2b_sb[:], in_=w2b_h[:])
        wz_sb = const.tile([64, B, 64], BF16)
        nc.sync.dma_start(out=wz_sb[:], in_=wz_h[:])
        fib_bc = const.tile([128, D], F32)
        nc.sync.dma_start(out=fib_bc[:], in_=fib_h[:].to_broadcast([128, D]))
        lng_bc = const.tile([128, D], F32)
        nc.sync.dma_start(out=lng_bc[:], in_=lng_h[:].to_broadcast([128, D]))
        lnb_bc = const.tile([128, D], F32)
        nc.sync.dma_start(out=lnb_bc[:], in_=lnb_h[:].to_broadcast([128, D]))
        eps_sb = const.tile([128, 1], F32)
        nc.vector.memset(eps_sb[:], LN_EPS)

        # long-lived activations (reserved early; lifetimes span several phases)
        late = ctx.enter_context(tc.tile_pool(name="late", bufs=1))
        xbf_sb = late.tile([64, NG, NTOK], BF16)     # final-x bf16 (written in pass r=2)
        yn_all = late.tile([128, NTT, D], BF16)      # post-LN, token-major
        ynT = late.tile([128, 8, NTOK], BF16)        # post-LN, feature-major

        with tc.tile_pool(name="xmaster", bufs=1) as xpool:
            x_sb = xpool.tile([64, NG, NTOK], BF16)  # bf16 master, per-group [64, 512]

            # ---- Phase 1: embedding gather + pos add + transpose to feature-major
            with tc.tile_pool(name="ph1", bufs=2) as ph1, \
                 tc.tile_pool(name="ph1xt", bufs=1) as ph1xt, \
                 tc.tile_pool(name="ph1w", bufs=1) as ph1w, \
                 tc.tile_pool(name="ph1ps", bufs=2, space="PSUM") as ph1ps, \
                 tc.tile_pool(name="fops", bufs=4, space="PSUM") as fops:
                xT = ph1xt.tile([128, 8, NTOK], BF16)    # x^T: [d-part, d-tile, tok]
                # single gather target: SWDGE pseudo-DMAs only support one sync
                # wait, so avoid slot-reuse WAR deps on the gather output
                g_all = ph1xt.tile([128, NTT, D], BF16)
                for tb in range(NTT):
                    nc.gpsimd.indirect_dma_start(
                        out=g_all[:, tb, :],
                        out_offset=None,
                        in_=emb_h[:],
                        in_offset=bass.IndirectOffsetOnAxis(ap=ids_sb[:, tb:tb + 1], axis=0),
                    )
                for tb in range(NTT):
                    p_tile = ph1.tile([128, D], F32)
                    nc.sync.dma_start(out=p_tile[:], in_=pos_h[tb * 128:(tb + 1) * 128, :])
                    xe = ph1.tile([128, D], F32)
                    nc.scalar.copy(out=xe[:], in_=g_all[:, tb, :])
                    xtok = ph1.tile([128, D], BF16)
                    nc.vector.tensor_add(out=xtok[:], in0=xe[:], in1=p_tile[:])
                    for dt in range(8):
                        tp = ph1ps.tile([128, 128], BF16)
                        nc.tensor.transpose(out=tp[:], in_=xtok[:, dt * 128:(dt + 1) * 128],
                                            identity=ident[:])
                        nc.scalar.copy(out=xT[:, dt, tb * 128:(tb + 1) * 128], in_=tp[:])

                # ---- Phase 2: fan_out  x_g = x @ fo_w[:, g] + fo_b[g]
                for half in range(2):
                    fow_sb = ph1w.tile([128, 8, T // 2], BF16)
                    nc.sync.dma_start(out=fow_sb[:],
                                      in_=fow_h[:, :, half * (T // 2):(half + 1) * (T // 2)])
                    for lg in range(NG // 2):
                        g = half * (NG // 2) + lg
                        ps = fops.tile([64, NTOK], F32)
                        for kt in range(8):
                            nc.tensor.matmul(
                                out=ps[:],
                                lhsT=fow_sb[:, kt, lg * 64:(lg + 1) * 64],
                                rhs=xT[:, kt, :],
                                start=(kt == 0), stop=(kt == 7),
                            )
                        nc.vector.tensor_scalar(
                            out=x_sb[:, g, :], in0=ps[:],
                            scalar1=fob_sb[:, g:g + 1], scalar2=None, op0=ALU.add,
                        )

            # ---- Phase 3: refine passes
            with tc.tile_pool(name="zstate", bufs=1) as zstate, \
                 tc.tile_pool(name="zp", bufs=2) as zp, \
                 tc.tile_pool(name="zbfp", bufs=2) as zbfp, \
                 tc.tile_pool(name="hbp", bufs=4) as hbp, \
                 tc.tile_pool(name="hps", bufs=4, space="PSUM") as hps, \
                 tc.tile_pool(name="ups", bufs=2, space="PSUM") as ups, \
                 tc.tile_pool(name="zps", bufs=2, space="PSUM") as zps:
                # zhat4 holds 4*z_hat (Wz is pre-scaled by 4 host-side) so the
                # pcm diff can use the un-normalized group sum directly.
                zhat4 = zstate.tile([64, B, NTOK], F32)
                acc1 = zstate.tile([64, B], F32)
                acc2 = zstate.tile([64, B], F32)
                nc.vector.memset(acc1[:], 0.0)
                nc.vector.memset(acc2[:], 0.0)
                for r in range(R):
                    # z-path (reads pre-update x); pairwise same-dtype adds
                    for b in range(B):
                        za = zp.tile([64, NTOK], F32)
                        nc.vector.tensor_add(out=za[:], in0=x_sb[:, 4 * b, :],
                                             in1=x_sb[:, 4 * b + 1, :])
                        zc = zp.tile([64, NTOK], F32)
                        nc.vector.tensor_add(out=zc[:], in0=x_sb[:, 4 * b + 2, :],
                                             in1=x_sb[:, 4 * b + 3, :])
                        zsum = zp.tile([64, NTOK], F32)
                        nc.vector.tensor_add(out=zsum[:], in0=za[:], in1=zc[:])
                        if r >= 1:
                            accr = acc1 if r == 1 else acc2
                            dd = zp.tile([64, NTOK], F32)
                            nc.vector.tensor_tensor(out=dd[:], in0=zsum[:], in1=zhat4[:, b, :],
                                                    op=ALU.subtract)
                            dd2 = zp.tile([64, NTOK], F32)
                            nc.scalar.activation(out=dd2[:], in_=dd[:], func=AF.Square,
                                                 accum_out=accr[:, b:b + 1])
                        if r <= 1:
                            zb = zbfp.tile([64, NTOK], BF16)
                            nc.scalar.mul(out=zb[:], in_=zsum[:], mul=0.25)
                            zph = zps.tile([64, NTOK], F32)
                            nc.tensor.matmul(out=zph[:], lhsT=wz_sb[:, b, :], rhs=zb[:],
                                             start=True, stop=True)
                            nc.scalar.copy(out=zhat4[:, b, :], in_=zph[:])
                    # MLP path, software-pipelined one group ahead so the PE
                    # never waits on gelu(g) before W2(g): W1(g+1) fills the gap.
                    w2_sb = w2a_sb if r == 0 else w2b_sb

                    def issue_w1(g):
                        b = g // 4
                        hts = []
                        for hh in range(2):
                            hp = hps.tile([128, NTOK], F32)
                            nc.tensor.matmul(out=hp[:],
                                             lhsT=w1_sb[:, b, hh * 128:(hh + 1) * 128],
                                             rhs=x_sb[:, g, :], start=True, stop=True)
                            ht = hbp.tile([128, NTOK], BF16)
                            nc.scalar.activation(out=ht[:], in_=hp[:], func=AF.Gelu_apprx_tanh)
                            hts.append(ht)
                        return hts

                    pend = issue_w1(0)
                    for g in range(NG):
                        b = g // 4
                        hts = pend
                        if g + 1 < NG:
                            pend = issue_w1(g + 1)
                        up = ups.tile([64, NTOK], F32)
                        for kk in range(2):
                            nc.tensor.matmul(out=up[:], lhsT=w2_sb[:, b, kk, :], rhs=hts[kk][:],
                                             start=(kk == 0), stop=(kk == 1))
                        if r < R - 1:
                            nc.vector.tensor_add(out=x_sb[:, g, :], in0=x_sb[:, g, :], in1=up[:])
                        else:
                            nc.vector.tensor_add(out=xbf_sb[:, g, :], in0=x_sb[:, g, :], in1=up[:])
                nc.sync.dma_start(out=aux_h[0], in_=acc1[:])
                nc.sync.dma_start(out=aux_h[1], in_=acc2[:])

        # x_sb freed here (xpool closed)

        # ---- Phase 4+5: fan_in + LayerNorm + transpose, interleaved per token
        # tile so the LN chain for tt hides under fan_in matmuls of tt+1 and
        # the PE only waits on one exposed LN chain (the last).
        with tc.tile_pool(name="fiw", bufs=1) as fiwp, \
             tc.tile_pool(name="yp", bufs=1) as yp, \
             tc.tile_pool(name="lnp", bufs=4) as lnp, \
             tc.tile_pool(name="fips", bufs=4, space="PSUM") as fips, \
             tc.tile_pool(name="tp2", bufs=2, space="PSUM") as tp2:
            fiw_sb = fiwp.tile([64, NG, D], BF16)
            nc.sync.dma_start(out=fiw_sb[:], in_=fiw_h[:])
            y_all = yp.tile([128, NTT, D], F32)
            nbs = nc.vector.BN_STATS_DIM

            def trans_yn(tt):
                for dt in range(8):
                    tp = tp2.tile([128, 128], BF16)
                    nc.tensor.transpose(out=tp[:], in_=yn_all[:, tt, dt * 128:(dt + 1) * 128],
                                        identity=ident[:])
                    nc.scalar.copy(out=ynT[:, dt, tt * 128:(tt + 1) * 128], in_=tp[:])

            for tt in range(NTT):
                for half in range(2):
                    ps = fips.tile([128, 512], F32)
                    for g in range(NG):
                        nc.tensor.matmul(
                            out=ps[:],
                            lhsT=xbf_sb[:, g, tt * 128:(tt + 1) * 128],
                            rhs=fiw_sb[:, g, half * 512:(half + 1) * 512],
                            start=(g == 0), stop=(g == NG - 1),
                        )
                    nc.vector.tensor_add(
                        out=y_all[:, tt, half * 512:(half + 1) * 512],
                        in0=ps[:], in1=fib_bc[:, half * 512:(half + 1) * 512],
                    )
                st = lnp.tile([128, 2, nbs], F32)
                nc.vector.bn_stats(out=st[:, 0, :], in_=y_all[:, tt, 0:512])
                nc.vector.bn_stats(out=st[:, 1, :], in_=y_all[:, tt, 512:1024])
                mv = lnp.tile([128, nc.vector.BN_AGGR_DIM], F32)
                nc.vector.bn_aggr(out=mv[:], in_=st[:])
                sd = lnp.tile([128, 1], F32)
                nc.scalar.activation(out=sd[:], in_=mv[:, 1:2], func=AF.Sqrt, bias=eps_sb[:])
                rstd = lnp.tile([128, 1], F32)
                nc.vector.reciprocal(out=rstd[:], in_=sd[:])
                tmp = lnp.tile([128, D], F32)
                nc.vector.tensor_scalar(out=tmp[:], in0=y_all[:, tt, :],
                                        scalar1=mv[:, 0:1], scalar2=rstd[:],
                                        op0=ALU.subtract, op1=ALU.mult)
                nc.vector.tensor_mul(out=tmp[:], in0=tmp[:], in1=lng_bc[:])
                nc.vector.tensor_add(out=yn_all[:, tt, :], in0=tmp[:], in1=lnb_bc[:])
                if tt >= 1:
                    trans_yn(tt - 1)
            trans_yn(NTT - 1)

        # ---- Phase 6: lm_head  logits = yn @ E^T
        et_ap = et_h[:].rearrange("(dt p) v -> p dt v", p=128)
        with tc.tile_pool(name="slab", bufs=3) as slabp, \
             tc.tile_pool(name="lst", bufs=4) as lstp, \
             tc.tile_pool(name="lmps", bufs=4, space="PSUM") as lmps:
            for vc in range(NVC):
                slab = slabp.tile([128, 8, VCH], BF16)
                nc.sync.dma_start(out=slab[:], in_=et_ap[:, :, vc * VCH:(vc + 1) * VCH])
                for tt in range(NTT):
                    ps = lmps.tile([128, VCH], F32)
                    for dt in range(8):
                        nc.tensor.matmul(
                            out=ps[:],
                            lhsT=ynT[:, dt, tt * 128:(tt + 1) * 128],
                            rhs=slab[:, dt, :],
                            start=(dt == 0), stop=(dt == 7),
                        )
                    ls = lstp.tile([128, VCH], F32)
                    nc.scalar.copy(out=ls[:], in_=ps[:])
                    nc.scalar.dma_start(
                        out=logits_h[tt * 128:(tt + 1) * 128, vc * VCH:(vc + 1) * VCH],
                        in_=ls[:],
                    )
    nc.finalize()
    return nc


def _get_program():
    global _PROGRAM
    if _PROGRAM is None:
        _PROGRAM = _build()
    return _PROGRAM


def kernel(**inputs):
    global LAST_RESULT
    nc = _get_program()

    ids = np.ascontiguousarray(np.asarray(inputs["input_ids"], np.int32).reshape(-1))
    emb = np.asarray(inputs["embedding"], np.float32)
    pos = np.asarray(inputs["pos_emb"], np.float32)
    lam = float(1.0 / (1.0 + np.exp(-np.float64(inputs["lambda_logit"]))))

    emb_bf = np.ascontiguousarray(emb.astype(BF))
    et_bf = np.ascontiguousarray(emb.T).astype(BF)
    fo_w = np.ascontiguousarray(
        np.asarray(inputs["fo_w"], np.float32).reshape(8, 128, T).transpose(1, 0, 2)
    ).astype(BF)
    fo_b = np.ascontiguousarray(
        np.asarray(inputs["fo_b"], np.float32).reshape(NG, 64).T
    )
    w1 = np.ascontiguousarray(
        np.asarray(inputs["W1"], np.float32).transpose(1, 0, 2)
    ).astype(BF)
    w2f = np.asarray(inputs["W2"], np.float32)
    w2a = np.ascontiguousarray(
        w2f.reshape(B, 2, 128, 64).transpose(2, 0, 1, 3)
    ).astype(BF)
    w2b = np.ascontiguousarray(
        (lam * w2f).reshape(B, 2, 128, 64).transpose(2, 0, 1, 3)
    ).astype(BF)
    wz = np.ascontiguousarray(
        (4.0 * np.asarray(inputs["Wz"], np.float32)).transpose(1, 0, 2)
    ).astype(BF)
    fi_w = np.ascontiguousarray(
        np.asarray(inputs["fi_w"], np.float32).reshape(NG, 64, D).transpose(1, 0, 2)
    ).astype(BF)
    fi_b = np.ascontiguousarray(np.asarray(inputs["fi_b"], np.float32).reshape(1, D))
    ln_g = np.ascontiguousarray(np.asarray(inputs["ln_g"], np.float32).reshape(1, D))
    ln_b = np.ascontiguousarray(np.asarray(inputs["ln_b"], np.float32).reshape(1, D))

    in_maps = []
    for i in range(NCORES):
        ids_i = np.ascontiguousarray(
            ids[i * NTOK:(i + 1) * NTOK].reshape(NTT, 128).T
        )
        p0 = (i * NTOK) % N
        pos_i = np.ascontiguousarray(pos[p0:p0 + NTOK])
        in_maps.append({
            "ids": ids_i, "emb": emb_bf, "et": et_bf, "pos": pos_i,
            "fo_w": fo_w, "fo_b": fo_b, "w1": w1, "w2a": w2a, "w2b": w2b,
            "wz": wz, "fi_w": fi_w, "fi_b": fi_b, "ln_g": ln_g, "ln_b": ln_b,
        })

    br = bass_utils.run_bass_kernel_spmd(nc, in_maps, core_ids=list(range(NCORES)))
    LAST_RESULT = br

    logits = np.concatenate(
        [np.asarray(r["logits"], np.float32) for r in br.results], axis=0
    ).reshape(BS, N, V)
    s = np.float64(0.0)
    for r in br.results:
        s += np.asarray(r["aux"], np.float64).sum()
    # acc holds sum((4z - 4zhat)^2) = 16 * sum((z - zhat)^2)
    aux = np.float32(PCM_W * (s / 16.0) / (BS * N * Dc))
    return logits, aux


# revision 28
# speedup vs baseline: 1.0585x; 1.0514x over previous
"""Trainium2 Bass kernel for NeuromorphicLM (8 NeuronCores, token-sharded SPMD).

Sharding: the 4096 tokens (BS*N flattened) are split into 8 contiguous shards of
512 tokens; every core runs the full pipeline (embed gather -> fan_out -> 3
refine passes -> fan_in -> LayerNorm -> tied lm_head over the whole 32k vocab)
on its shard. No collectives. Host gathers logits shards and sums pcm partials.
"""
import sys

sys.path.insert(0, "/opt/trn_rl_repo")

from contextlib import ExitStack

import numpy as np
import ml_dtypes

import concourse.bass as bass
import concourse.bacc as bacc
import concourse.tile as tile
from concourse import mybir
from concourse import bass_utils
from concourse.masks import make_identity

BF = ml_dtypes.bfloat16
F32 = mybir.dt.float32
BF16 = mybir.dt.bfloat16

BS, N, V, D = 4, 1024, 32000, 1024
B, C, Dc, H, R = 8, 4, 64, 256, 3
T = B * C * Dc            # 2048
NG = B * C                # 32 (block, channel) groups of 64 features
NCORES = 8
NTOK = (BS * N) // NCORES  # 512 tokens per core
NTT = NTOK // 128          # 4 token tiles of 128
VCH = 500                  # vocab chunk (lm_head matmul N)
NVC = V // VCH             # 64 chunks
PCM_W = 0.1
LN_EPS = 1e-5

AF = mybir.ActivationFunctionType
ALU = mybir.AluOpType

_PROGRAM = None
LAST_RESULT = None


def _build():
    # Bacc (not plain Bass): its finalize() runs the backend passes, notably
    # generate_event_semaphores which splits multi-sem waits (TRN2 allows one
    # sync wait per instruction).
    nc = bacc.Bacc("TRN2")

    ids_h = nc.declare_dram_parameter("ids", [128, NTT], mybir.dt.int32, isOutput=False)
    emb_h = nc.declare_dram_parameter("emb", [V, D], BF16, isOutput=False)
    et_h = nc.declare_dram_parameter("et", [D, V], BF16, isOutput=False)
    pos_h = nc.declare_dram_parameter("pos", [NTOK, D], F32, isOutput=False)
    fow_h = nc.declare_dram_parameter("fo_w", [128, 8, T], BF16, isOutput=False)
    fob_h = nc.declare_dram_parameter("fo_b", [64, NG], F32, isOutput=False)
    w1_h = nc.declare_dram_parameter("w1", [64, B, H], BF16, isOutput=False)
    w2a_h = nc.declare_dram_parameter("w2a", [128, B, 2, 64], BF16, isOutput=False)
    w2b_h = nc.declare_dram_parameter("w2b", [128, B, 2, 64], BF16, isOutput=False)
    wz_h = nc.declare_dram_parameter("wz", [64, B, 64], BF16, isOutput=False)
    fiw_h = nc.declare_dram_parameter("fi_w", [64, NG, D], BF16, isOutput=False)
    fib_h = nc.declare_dram_parameter("fi_b", [1, D], F32, isOutput=False)
    lng_h = nc.declare_dram_parameter("ln_g", [1, D], F32, isOutput=False)
    lnb_h = nc.declare_dram_parameter("ln_b", [1, D], F32, isOutput=False)
    logits_h = nc.declare_dram_parameter("logits", [NTOK, V], F32, isOutput=True)
    aux_h = nc.declare_dram_parameter("aux", [2, 64, B], F32, isOutput=True)

    with tile.TileContext(nc) as tc, ExitStack() as ctx:
        const = ctx.enter_context(tc.tile_pool(name="const", bufs=1))
        ident = const.tile([128, 128], BF16)
        make_identity(nc, ident[:])

        ids_sb = const.tile([128, NTT], mybir.dt.int32)
        nc.sync.dma_start(out=ids_sb[:], in_=ids_h[:])
        fob_sb = const.tile([64, NG], F32)
        nc.sync.dma_start(out=fob_sb[:], in_=fob_h[:])
        w1_sb = const.tile([64, B, H], BF16)
        nc.sync.dma_start(out=w1_sb[:], in_=w1_h[:])
        w2a_sb = const.tile([128, B, 2, 64], BF16)
        nc.sync.dma_start(out=w2a_sb[:], in_=w2a_h[:])
        w2b_sb = const.tile([128, B, 2, 64], BF16)
        nc.sync.dma_start(out=w2b_sb[:], in_=w2b_h[:])
        wz_sb = const.tile([64, B, 64], BF16)
        nc.sync.dma_start(out=wz_sb[:], in_=wz_h[:])
        fib_bc = const.tile([128, D], F32)
        nc.sync.dma_start(out=fib_bc[:], in_=fib_h[:].to_broadcast([128, D]))
        lng_bc = const.tile([128, D], F32)
        nc.sync.dma_start(out=lng_bc[:], in_=lng_h[:].to_broadcast([128, D]))
        lnb_bc = const.tile([128, D], F32)
        nc.sync.dma_start(out=lnb_bc[:], in_=lnb_h[:].to_broadcast([128, D]))
        eps_sb = const.tile([128, 1], F32)
        nc.vector.memset(eps_sb[:], LN_EPS)

        # long-lived activations (reserved early; lifetimes span several phases)
        late = ctx.enter_context(tc.tile_pool(name="late", bufs=1))
        xbf_sb = late.tile([64, NG, NTOK], BF16)     # final-x bf16 (written in pass r=2)
        yn_all = late.tile([128, NTT, D], BF16)      # post-LN, token-major
        ynT = late.tile([128, 8, NTOK], BF16)        # post-LN, feature-major

        with tc.tile_pool(name="xmaster", bufs=1) as xpool:
            x_sb = xpool.tile([64, NG, NTOK], BF16)  # bf16 master, per-group [64, 512]

            # ---- Phase 1: embedding gather + pos add + transpose to feature-major
            with tc.tile_pool(name="ph1", bufs=2) as ph1, \
                 tc.tile_pool(name="ph1xt", bufs=1) as ph1xt, \
                 tc.tile_pool(name="ph1w", bufs=1) as ph1w, \
                 tc.tile_pool(name="ph1ps", bufs=2, space="PSUM") as ph1ps, \
                 tc.tile_pool(name="fops", bufs=4, space="PSUM") as fops:
                xT = ph1xt.tile([128, 8, NTOK], BF16)    # x^T: [d-part, d-tile, tok]
                # single gather target: SWDGE pseudo-DMAs only support one sync
                # wait, so avoid slot-reuse WAR deps on the gather output
                g_all = ph1xt.tile([128, NTT, D], BF16)
                for tb in range(NTT):
                    nc.gpsimd.indirect_dma_start(
                        out=g_all[:, tb, :],
                        out_offset=None,
                        in_=emb_h[:],
                        in_offset=bass.IndirectOffsetOnAxis(ap=ids_sb[:, tb:tb + 1], axis=0),
                    )
                for tb in range(NTT):
                    p_tile = ph1.tile([128, D], F32)
                    nc.sync.dma_start(out=p_tile[:], in_=pos_h[tb * 128:(tb + 1) * 128, :])
                    xe = ph1.tile([128, D], F32)
                    nc.scalar.copy(out=xe[:], in_=g_all[:, tb, :])
                    xtok = ph1.tile([128, D], BF16)
                    nc.vector.tensor_add(out=xtok[:], in0=xe[:], in1=p_tile[:])
                    for dt in range(8):
                        tp = ph1ps.tile([128, 128], BF16)
                        nc.tensor.transpose(out=tp[:], in_=xtok[:, dt * 128:(dt + 1) * 128],
                                            identity=ident[:])
                        nc.scalar.copy(out=xT[:, dt, tb * 128:(tb + 1) * 128], in_=tp[:])

                # ---- Phase 2: fan_out  x_g = x @ fo_w[:, g] + fo_b[g]
                for half in range(2):
                    fow_sb = ph1w.tile([128, 8, T // 2], BF16)
                    nc.sync.dma_start(out=fow_sb[:],
                                      in_=fow_h[:, :, half * (T // 2):(half + 1) * (T // 2)])
                    for lg in range(NG // 2):
                        g = half * (NG // 2) + lg
                        ps = fops.tile([64, NTOK], F32)
                        for kt in range(8):
                            nc.tensor.matmul(
                                out=ps[:],
                                lhsT=fow_sb[:, kt, lg * 64:(lg + 1) * 64],
                                rhs=xT[:, kt, :],
                                start=(kt == 0), stop=(kt == 7),
                            )
                        nc.vector.tensor_scalar(
                            out=x_sb[:, g, :], in0=ps[:],
                            scalar1=fob_sb[:, g:g + 1], scalar2=None, op0=ALU.add,
                        )

            # ---- Phase 3: refine passes
            with tc.tile_pool(name="zstate", bufs=1) as zstate, \
                 tc.tile_pool(name="zp", bufs=2) as zp, \
                 tc.tile_pool(name="zbfp", bufs=2) as zbfp, \
                 tc.tile_pool(name="hbp", bufs=4) as hbp, \
                 tc.tile_pool(name="hps", bufs=4, space="PSUM") as hps, \
                 tc.tile_pool(name="ups", bufs=2, space="PSUM") as ups, \
                 tc.tile_pool(name="zps", bufs=2, space="PSUM") as zps:
                # zhat4 holds 4*z_hat (Wz is pre-scaled by 4 host-side) so the
                # pcm diff can use the un-normalized group sum directly.
                zhat4 = zstate.tile([64, B, NTOK], F32)
                acc1 = zstate.tile([64, B], F32)
                acc2 = zstate.tile([64, B], F32)
                nc.vector.memset(acc1[:], 0.0)
                nc.vector.memset(acc2[:], 0.0)
                for r in range(R):
                    # z-path (reads pre-update x); pairwise same-dtype adds
                    for b in range(B):
                        za = zp.tile([64, NTOK], F32)
                        nc.vector.tensor_add(out=za[:], in0=x_sb[:, 4 * b, :],
                                             in1=x_sb[:, 4 * b + 1, :])
                        zc = zp.tile([64, NTOK], F32)
                        nc.vector.tensor_add(out=zc[:], in0=x_sb[:, 4 * b + 2, :],
                                             in1=x_sb[:, 4 * b + 3, :])
                        zsum = zp.tile([64, NTOK], F32)
                        nc.vector.tensor_add(out=zsum[:], in0=za[:], in1=zc[:])
                        if r >= 1:
                            accr = acc1 if r == 1 else acc2
                            dd = zp.tile([64, NTOK], F32)
                            nc.vector.tensor_tensor(out=dd[:], in0=zsum[:], in1=zhat4[:, b, :],
                                                    op=ALU.subtract)
                            dd2 = zp.tile([64, NTOK], F32)
                            nc.scalar.activation(out=dd2[:], in_=dd[:], func=AF.Square,
                                                 accum_out=accr[:, b:b + 1])
                        if r <= 1:
                            zb = zbfp.tile([64, NTOK], BF16)
                            nc.scalar.mul(out=zb[:], in_=zsum[:], mul=0.25)
                            zph = zps.tile([64, NTOK], F32)
                            nc.tensor.matmul(out=zph[:], lhsT=wz_sb[:, b, :], rhs=zb[:],
                                             start=True, stop=True)
                            nc.scalar.copy(out=zhat4[:, b, :], in_=zph[:])
                    # MLP path, software-pipelined one group ahead so the PE
                    # never waits on gelu(g) before W2(g): W1(g+1) fills the gap.
                    w2_sb = w2a_sb if r == 0 else w2b_sb

                    def issue_w1(g):
                        b = g // 4
                        hts = []
                        for hh in range(2):
                            hp = hps.tile([128, NTOK], F32)
                            nc.tensor.matmul(out=hp[:],
                                             lhsT=w1_sb[:, b, hh * 128:(hh + 1) * 128],
                                             rhs=x_sb[:, g, :], start=True, stop=True)
                            ht = hbp.tile([128, NTOK], BF16)
                            nc.scalar.activation(out=ht[:], in_=hp[:], func=AF.Gelu_apprx_tanh)
                            hts.append(ht)
                        return hts

                    pend = issue_w1(0)
                    for g in range(NG):
                        b = g // 4
                        hts = pend
                        if g + 1 < NG:
                            pend = issue_w1(g + 1)
                        up = ups.tile([64, NTOK], F32)
                        for kk in range(2):
                            nc.tensor.matmul(out=up[:], lhsT=w2_sb[:, b, kk, :], rhs=hts[kk][:],
                                             start=(kk == 0), stop=(kk == 1))
                        if r < R - 1:
                            nc.vector.tensor_add(out=x_sb[:, g, :], in0=x_sb[:, g, :], in1=up[:])
                        else:
                            nc.vector.tensor_add(out=xbf_sb[:, g, :], in0=x_sb[:, g, :], in1=up[:])
                # scalar queue (idle here) so the sync queue stays clear
                nc.scalar.dma_start(out=aux_h[0], in_=acc1[:])
                nc.scalar.dma_start(out=aux_h[1], in_=acc2[:])

        # x_sb freed here (xpool closed)

        # ---- Phase 4+5: fan_in + LayerNorm + transpose, interleaved per token
        # tile so the LN chain for tt hides under fan_in matmuls of tt+1 and
        # the PE only waits on one exposed LN chain (the last).
        with tc.tile_pool(name="fiw", bufs=1) as fiwp, \
             tc.tile_pool(name="yp", bufs=1) as yp, \
             tc.tile_pool(name="lnp", bufs=4) as lnp, \
             tc.tile_pool(name="fips", bufs=4, space="PSUM") as fips, \
             tc.tile_pool(name="tp2", bufs=2, space="PSUM") as tp2:
            # two half-loads: the transfer is WAR-gated on refine-era SBUF, so
            # splitting lets the first fan_in matmuls start after only 2MB
            fiw0 = fiwp.tile([64, NG, D // 2], BF16)
            fiw1 = fiwp.tile([64, NG, D // 2], BF16)
            fiw_half = [fiw0, fiw1]
            for h in range(2):
                nc.sync.dma_start(out=fiw_half[h][:],
                                  in_=fiw_h[:, :, h * 512:(h + 1) * 512])
            y_all = yp.tile([128, NTT, D], F32)
            nbs = nc.vector.BN_STATS_DIM

            def trans_yn(tt):
                for dt in range(8):
                    tp = tp2.tile([128, 128], BF16)
                    nc.tensor.transpose(out=tp[:], in_=yn_all[:, tt, dt * 128:(dt + 1) * 128],
                                        identity=ident[:])
                    nc.scalar.copy(out=ynT[:, dt, tt * 128:(tt + 1) * 128], in_=tp[:])

            for tt in range(NTT):
                for half in range(2):
                    ps = fips.tile([128, 512], F32)
                    for g in range(NG):
                        nc.tensor.matmul(
                            out=ps[:],
                            lhsT=xbf_sb[:, g, tt * 128:(tt + 1) * 128],
                            rhs=fiw_half[half][:, g, :],
                            start=(g == 0), stop=(g == NG - 1),
                        )
                    nc.vector.tensor_add(
                        out=y_all[:, tt, half * 512:(half + 1) * 512],
                        in0=ps[:], in1=fib_bc[:, half * 512:(half + 1) * 512],
                    )
                st = lnp.tile([128, 2, nbs], F32)
                nc.vector.bn_stats(out=st[:, 0, :], in_=y_all[:, tt, 0:512])
                nc.vector.bn_stats(out=st[:, 1, :], in_=y_all[:, tt, 512:1024])
                mv = lnp.tile([128, nc.vector.BN_AGGR_DIM], F32)
                nc.vector.bn_aggr(out=mv[:], in_=st[:])
                sd = lnp.tile([128, 1], F32)
                nc.scalar.activation(out=sd[:], in_=mv[:, 1:2], func=AF.Sqrt, bias=eps_sb[:])
                rstd = lnp.tile([128, 1], F32)
                nc.vector.reciprocal(out=rstd[:], in_=sd[:])
                tmp = lnp.tile([128, D], F32)
                nc.vector.tensor_scalar(out=tmp[:], in0=y_all[:, tt, :],
                                        scalar1=mv[:, 0:1], scalar2=rstd[:],
                                        op0=ALU.subtract, op1=ALU.mult)
                nc.vector.tensor_mul(out=tmp[:], in0=tmp[:], in1=lng_bc[:])
                nc.vector.tensor_add(out=yn_all[:, tt, :], in0=tmp[:], in1=lnb_bc[:])
                if tt >= 1:
                    trans_yn(tt - 1)
            trans_yn(NTT - 1)

        # ---- Phase 6: lm_head  logits = yn @ E^T
        et_ap = et_h[:].rearrange("(dt p) v -> p dt v", p=128)
        with tc.tile_pool(name="slab", bufs=3) as slabp, \
             tc.tile_pool(name="lst", bufs=4) as lstp, \
             tc.tile_pool(name="lmps", bufs=4, space="PSUM") as lmps:
            for vc in range(NVC):
                slab = slabp.tile([128, 8, VCH], BF16)
                nc.sync.dma_start(out=slab[:], in_=et_ap[:, :, vc * VCH:(vc + 1) * VCH])
                for tt in range(NTT):
                    ps = lmps.tile([128, VCH], F32)
                    for dt in range(8):
                        nc.tensor.matmul(
                            out=ps[:],
                            lhsT=ynT[:, dt, tt * 128:(tt + 1) * 128],
                            rhs=slab[:, dt, :],
                            start=(dt == 0), stop=(dt == 7),
                        )
                    ls = lstp.tile([128, VCH], F32)
                    nc.scalar.copy(out=ls[:], in_=ps[:])
                    # alternate output queues so the 64MB logits writeback
                    # drains on two DMA rings instead of one
                    eng = nc.scalar if (vc * NTT + tt) % 2 == 0 else nc.gpsimd
                    eng.dma_start(
                        out=logits_h[tt * 128:(tt + 1) * 128, vc * VCH:(vc + 1) * VCH],
                        in_=ls[:],
                    )
    nc.finalize()
    return nc


def _get_program():
    global _PROGRAM
    if _PROGRAM is None:
        _PROGRAM = _build()
    return _PROGRAM


def kernel(**inputs):
    global LAST_RESULT
    nc = _get_program()

    ids = np.ascontiguousarray(np.asarray(inputs["input_ids"], np.int32).reshape(-1))
    emb = np.asarray(inputs["embedding"], np.float32)
    pos = np.asarray(inputs["pos_emb"], np.float32)
    lam = float(1.0 / (1.0 + np.exp(-np.float64(inputs["lambda_logit"]))))

    emb_bf = np.ascontiguousarray(emb.astype(BF))
    et_bf = np.ascontiguousarray(emb.T).astype(BF)
    fo_w = np.ascontiguousarray(
        np.asarray(inputs["fo_w"], np.float32).reshape(8, 128, T).transpose(1, 0, 2)
    ).astype(BF)
    fo_b = np.ascontiguousarray(
        np.asarray(inputs["fo_b"], np.float32).reshape(NG, 64).T
    )
    w1 = np.ascontiguousarray(
        np.asarray(inputs["W1"], np.float32).transpose(1, 0, 2)
    ).astype(BF)
    w2f = np.asarray(inputs["W2"], np.float32)
    w2a = np.ascontiguousarray(
        w2f.reshape(B, 2, 128, 64).transpose(2, 0, 1, 3)
    ).astype(BF)
    w2b = np.ascontiguousarray(
        (lam * w2f).reshape(B, 2, 128, 64).transpose(2, 0, 1, 3)
    ).astype(BF)
    wz = np.ascontiguousarray(
        (4.0 * np.asarray(inputs["Wz"], np.float32)).transpose(1, 0, 2)
    ).astype(BF)
    fi_w = np.ascontiguousarray(
        np.asarray(inputs["fi_w"], np.float32).reshape(NG, 64, D).transpose(1, 0, 2)
    ).astype(BF)
    fi_b = np.ascontiguousarray(np.asarray(inputs["fi_b"], np.float32).reshape(1, D))
    ln_g = np.ascontiguousarray(np.asarray(inputs["ln_g"], np.float32).reshape(1, D))
    ln_b = np.ascontiguousarray(np.asarray(inputs["ln_b"], np.float32).reshape(1, D))

    in_maps = []
    for i in range(NCORES):
        ids_i = np.ascontiguousarray(
            ids[i * NTOK:(i + 1) * NTOK].reshape(NTT, 128).T
        )
        p0 = (i * NTOK) % N
        pos_i = np.ascontiguousarray(pos[p0:p0 + NTOK])
        in_maps.append({
            "ids": ids_i, "emb": emb_bf, "et": et_bf, "pos": pos_i,
            "fo_w": fo_w, "fo_b": fo_b, "w1": w1, "w2a": w2a, "w2b": w2b,
            "wz": wz, "fi_w": fi_w, "fi_b": fi_b, "ln_g": ln_g, "ln_b": ln_b,
        })

    br = bass_utils.run_bass_kernel_spmd(nc, in_maps, core_ids=list(range(NCORES)))
    LAST_RESULT = br

    logits = np.concatenate(
        [np.asarray(r["logits"], np.float32) for r in br.results], axis=0
    ).reshape(BS, N, V)
    s = np.float64(0.0)
    for r in br.results:
        s += np.asarray(r["aux"], np.float64).sum()
    # acc holds sum((4z - 4zhat)^2) = 16 * sum((z - zhat)^2)
    aux = np.float32(PCM_W * (s / 16.0) / (BS * N * Dc))
    return logits, aux
